# revision 9
# baseline (speedup 1.0000x reference)
"""MoE (top-2 of 8 experts, SwiGLU FFN) Trainium2 kernel.

Strategy (expert-parallel, per the sharding hint):
 - Host computes the router (logits, top-2, softmax weights, aux losses) --
   dispatch decides the sharding, so it lives with the host-side
   shard/unshard logic.
 - Each of the 8 NeuronCores owns one expert: it receives that expert's
   weights (pre-transposed, pre-blocked, bf16-cast on host) and the tokens
   routed to it (gathered + padded to a common capacity C), and computes
   y_e^T[D, C] = W2^T(e) @ ((W1(e) x^T) * silu(W3(e) x^T)).
 - Host scatter-adds coef_e * y_e back into the full [B,S,D] output (each
   token is claimed by exactly two experts).

Device kernel (per core), all matmuls bf16 with fp32 PSUM accumulation:
 - Phase 1: hgT[H, C] = (W1 x^T) * silu(W3 x^T). j-tiles of 128 rows of H,
   K=D contraction on partitions. W1T/W3T stream through SBUF in interleaved
   512-column blocks; x^T is resident; W2T prefetches behind the stream.
 - Phase 2: y^T[D, C] = W2T^T @ hgT with K=H contraction; D-tile on
   partitions, tokens on the free dim (no transposes anywhere; the token
   coef is applied on the host during the combine).
"""

import numpy as np
import ml_dtypes

import concourse.bacc as bacc
import concourse.mybir as mybir
import concourse.tile as tile
from concourse.bass_utils import run_bass_kernel_spmd

BF16 = ml_dtypes.bfloat16

DIM = 1024
HID = 4096
E = 8
TOP_K = 2
Z_LOSS_COEF = 0.001
NCORES = 8

P = 128
KD = DIM // P        # 8  k-chunks over D
KH = HID // P        # 32 k-chunks over H
JB = 4               # j-tiles (128 cols of 2H) per streamed weight block
NPAIRS = JB // 2
GROUPS = 2 * HID // (P * JB)  # 16 blocks covering [W1T | W3T] interleaved

_compiled = {}
_weight_cache = {"key": None, "packed": None}


def _chunks(total, step):
    out, c0 = [], 0
    while c0 < total:
        out.append((c0, min(step, total - c0)))
        c0 += step
    return out


def _build(C, reps=1):
    """Build + compile the per-core Bass kernel for token capacity C.

    reps>1 repeats the compute body (benchmarking only: wall-clock slope
    over reps isolates device time from transfer/dispatch overhead)."""
    assert C % 32 == 0
    nc = bacc.Bacc("TRN2", target_bir_lowering=False, debug=False,
                   num_devices=NCORES)
    dt = mybir.dt
    wA = nc.dram_tensor("wA", [GROUPS, P, KD, JB * P], dt.bfloat16,
                        kind="ExternalInput")
    w2 = nc.dram_tensor("w2", [P, KH, DIM], dt.bfloat16, kind="ExternalInput")
    xT = nc.dram_tensor("xT", [P, KD, C], dt.bfloat16, kind="ExternalInput")
    y = nc.dram_tensor("y", [DIM, C], dt.float32, kind="ExternalOutput")

    cchunks = _chunks(C, 512)

    with tile.TileContext(nc) as tc:
        with (
            tc.tile_pool(name="resident", bufs=1) as resident,
            tc.tile_pool(name="wstream", bufs=3) as wstream,
            tc.tile_pool(name="gtmp", bufs=2) as gpool,
            tc.tile_pool(name="yout", bufs=3) as ypool,
            tc.tile_pool(name="ps1", bufs=5, space="PSUM") as ps1,
            tc.tile_pool(name="ps2", bufs=3, space="PSUM") as ps2,
        ):
          for _rep in range(reps):
            xsb = resident.tile([P, KD, C], dt.bfloat16, tag="xsb")
            wsb0 = wstream.tile([P, KD, JB * P], dt.bfloat16, tag="wsb")
            # interleave the first weight block with x so the first matmul
            # group's inputs land as early as possible
            for k in range(KD):
                nc.sync.dma_start(wsb0[:, k, :], wA[0, :, k, :])
                nc.sync.dma_start(xsb[:, k, :], xT[:, k, :])
            hgsb = resident.tile([P, KH, C], dt.bfloat16, tag="hgsb")
            w2sb = resident.tile([P, KH, DIM], dt.bfloat16, tag="w2sb")

            # ---- phase 1: hgT[j*128+jj, c] = h * silu(g) ----
            w2_start = 2   # delay w2 prefetch so the wA stream stays ahead
            w2_per_g = (KH + GROUPS - w2_start - 1) // (GROUPS - w2_start)
            for g in range(GROUPS):
                if g == 0:
                    wsb = wsb0
                else:
                    wsb = wstream.tile([P, KD, JB * P], dt.bfloat16,
                                       tag="wsb")
                    nc.sync.dma_start(wsb[:], wA[g])
                # w2 prefetch rides behind the phase-1 weight stream
                if g >= w2_start:
                    k0 = (g - w2_start) * w2_per_g
                    for kk in range(k0, min(KH, k0 + w2_per_g)):
                        nc.sync.dma_start(w2sb[:, kk, :], w2[:, kk, :])
                if g == 0:
                    # k-outer over all 4 psum groups: each arriving k-chunk
                    # DMA feeds 8 matmuls, hiding the startup DMA cadence
                    for (c0, cn) in cchunks:
                        ps = [ps1.tile([P, cn], dt.float32, tag="ps_h",
                                       name=f"ps{i}")
                              for i in range(2 * NPAIRS)]
                        for k in range(KD):
                            for jt in range(NPAIRS):
                                nc.tensor.matmul(
                                    ps[2 * jt][:],
                                    wsb[:, k, jt * P:(jt + 1) * P],
                                    xsb[:, k, c0:c0 + cn],
                                    start=(k == 0), stop=(k == KD - 1))
                                nc.tensor.matmul(
                                    ps[2 * jt + 1][:],
                                    wsb[:, k,
                                        (NPAIRS + jt) * P:(NPAIRS + jt + 1) * P],
                                    xsb[:, k, c0:c0 + cn],
                                    start=(k == 0), stop=(k == KD - 1))
                        for jt in range(NPAIRS):
                            j = NPAIRS * g + jt
                            gt = gpool.tile([P, cn], dt.float32, tag="gt")
                            nc.scalar.activation(
                                gt[:], ps[2 * jt + 1][:],
                                mybir.ActivationFunctionType.Silu)
                            nc.vector.tensor_mul(
                                hgsb[:, j, c0:c0 + cn], ps[2 * jt][:], gt[:])
                    continue
                for jt in range(NPAIRS):
                    j = NPAIRS * g + jt
                    for (c0, cn) in cchunks:
                        ps_h = ps1.tile([P, cn], dt.float32, tag="ps_h")
                        ps_g = ps1.tile([P, cn], dt.float32, tag="ps_h")
                        for k in range(KD):
                            nc.tensor.matmul(
                                ps_h[:], wsb[:, k, jt * P:(jt + 1) * P],
                                xsb[:, k, c0:c0 + cn],
                                start=(k == 0), stop=(k == KD - 1))
                        for k in range(KD):
                            nc.tensor.matmul(
                                ps_g[:],
                                wsb[:, k,
                                    (NPAIRS + jt) * P:(NPAIRS + jt + 1) * P],
                                xsb[:, k, c0:c0 + cn],
                                start=(k == 0), stop=(k == KD - 1))
                        gt = gpool.tile([P, cn], dt.float32, tag="gt")
                        nc.scalar.activation(
                            gt[:], ps_g[:], mybir.ActivationFunctionType.Silu)
                        nc.vector.tensor_mul(
                            hgsb[:, j, c0:c0 + cn], ps_h[:], gt[:])

            # ---- phase 2: y^T[d, c] = sum_h w2T[h, d] * hgT[h, c] ----
            for dtile in range(DIM // P):
                for (c0, cn) in cchunks:
                    ps_y = ps2.tile([P, cn], dt.float32, tag="ps_y")
                    for kk in range(KH):
                        nc.tensor.matmul(
                            ps_y[:], w2sb[:, kk, dtile * P:(dtile + 1) * P],
                            hgsb[:, kk, c0:c0 + cn],
                            start=(kk == 0), stop=(kk == KH - 1))
                    ysb = ypool.tile([P, cn], dt.float32, tag="ysb")
                    nc.vector.tensor_copy(ysb[:], ps_y[:])
                    nc.sync.dma_start(
                        y[dtile * P:(dtile + 1) * P, c0:c0 + cn], ysb[:])

    nc.compile()
    return nc


def _route(x2d, Wr):
    """Host router: returns (top2 idx [T,2], top2 weights [T,2], z_loss,
    balance_loss). Mirrors the jax reference in fp32."""
    logits = (x2d @ Wr.T.astype(np.float32)).astype(np.float32)  # [T, E]
    order = np.argsort(-logits, axis=1, kind="stable")
    top_idx = order[:, :TOP_K]
    top_val = np.take_along_axis(logits, top_idx, axis=1)
    m = top_val.max(axis=1, keepdims=True)
    w = np.exp(top_val - m, dtype=np.float32)
    top_w = (w / w.sum(axis=1, keepdims=True)).astype(np.float32)

    z_loss = np.float32(np.mean(np.square(logits), dtype=np.float32)
                        * Z_LOSS_COEF)
    lm = logits.max(axis=1, keepdims=True)
    p = np.exp(logits - lm, dtype=np.float32)
    probs = p / p.sum(axis=1, keepdims=True)
    pmean = probs.mean(axis=0, dtype=np.float32)
    balance_loss = np.float32(
        np.mean(np.square(pmean - np.float32(1.0 / E)), dtype=np.float32))
    return top_idx, top_w, z_loss, balance_loss


def _pack_weights(W1e, W3e):
    """[H,D] fp32 pair -> blocked [GROUPS, P, KD, JB*P] bf16: group g holds
    j-tiles (2g, 2g+1) of W1^T then of W3^T, [p, k, jj] within a block."""
    A = np.ascontiguousarray(W1e.T).reshape(KD, P, KH, P)   # [k, p, j, jj]
    B = np.ascontiguousarray(W3e.T).reshape(KD, P, KH, P)
    Ag = A.reshape(KD, P, GROUPS, NPAIRS, P).transpose(2, 1, 0, 3, 4)
    Bg = B.reshape(KD, P, GROUPS, NPAIRS, P).transpose(2, 1, 0, 3, 4)
    blk = np.concatenate(
        [Ag.reshape(GROUPS, P, KD, NPAIRS * P),
         Bg.reshape(GROUPS, P, KD, NPAIRS * P)], axis=3)
    return np.ascontiguousarray(blk).astype(BF16)


def kernel(x, Wr, W1, W2, W3):
    x = np.asarray(x, dtype=np.float32)
    Wr = np.asarray(Wr, dtype=np.float32)
    W1 = np.asarray(W1, dtype=np.float32)
    W2 = np.asarray(W2, dtype=np.float32)
    W3 = np.asarray(W3, dtype=np.float32)

    Bb, S, D = x.shape
    T = Bb * S
    x2d = np.ascontiguousarray(x.reshape(T, D))

    top_idx, top_w, z_loss, balance_loss = _route(x2d, Wr)

    # dispatch lists per expert
    idx_e, w_e = [], []
    for e in range(E):
        hits = np.nonzero(top_idx == e)
        idx_e.append(hits[0].astype(np.int64))
        w_e.append(top_w[hits[0], hits[1]].astype(np.float32))
    counts = np.array([len(i) for i in idx_e])
    C = max(64, int(np.ceil(counts.max() / 32)) * 32)

    if C not in _compiled:
        _compiled[C] = _build(C)
    nc = _compiled[C]

    # weight packing is pure + deterministic in (W1, W2, W3): cache it
    if (_weight_cache["key"] is not None
            and all(np.array_equal(a, b) for a, b in
                    zip(_weight_cache["key"], (W1, W2, W3)))):
        packed = _weight_cache["packed"]
    else:
        packed = []
        for e in range(E):
            w2t = np.ascontiguousarray(W2[e].T)  # [H, D]
            w2blk = np.ascontiguousarray(
                w2t.reshape(KH, P, DIM).transpose(1, 0, 2)).astype(BF16)
            packed.append({"wA": _pack_weights(W1[e], W3[e]), "w2": w2blk})
        _weight_cache["key"] = (W1.copy(), W2.copy(), W3.copy())
        _weight_cache["packed"] = packed

    in_maps = []
    for e in range(E):
        n = counts[e]
        xg = np.zeros((C, D), dtype=np.float32)
        xg[:n] = x2d[idx_e[e]]
        # xTe[p, k, c] = xg[c, k*P + p]
        xTe = np.ascontiguousarray(xg.reshape(C, KD, P).transpose(2, 1, 0))
        in_maps.append({
            "wA": packed[e]["wA"],
            "w2": packed[e]["w2"],
            "xT": xTe.astype(BF16),
        })

    # the axon tunnel occasionally reports a transient device error; retry
    res = None
    for attempt in range(3):
        try:
            res = run_bass_kernel_spmd(nc, in_maps,
                                       core_ids=list(range(NCORES)))
            break
        except Exception:
            if attempt == 2:
                raise
            import time as _time
            _time.sleep(2.0)

    y_full = np.zeros((T, D), dtype=np.float32)
    for e in range(E):
        n = counts[e]
        if n:
            # y output is [D, C]; apply the top-2 softmax coef on combine
            y_full[idx_e[e]] += res.results[e]["y"][:, :n].T * w_e[e][:, None]

    return (y_full.reshape(Bb, S, D), z_loss, balance_loss)


# revision 10
# speedup vs baseline: 1.0271x; 1.0271x over previous
"""MoE kernel v2: pairwise H-sliced expert parallelism.

Core pairs (2p, 2p+1) jointly own two experts: each core holds H-rows
[(c%2)*2048, (c%2+1)*2048) of both experts' W1/W3 (and the matching W2
columns) and processes both experts' exact token lists. Per-core slot
capacities are (max big-expert, max small-expert) over pairs -- (544, 512)
here vs a uniform 544+... in the expert-per-core layout: PE columns drop
418k -> 405k and every core does identical work. The host sums each pair's
two partial y^T arrays (fp32) and scatter-adds with the routing coefs.
"""

import numpy as np
import ml_dtypes

import concourse.bacc as bacc
import concourse.mybir as mybir
import concourse.tile as tile
from concourse.bass_utils import run_bass_kernel_spmd

BF16 = ml_dtypes.bfloat16

DIM = 1024
HID = 4096
E = 8
TOP_K = 2
Z_LOSS_COEF = 0.001
NCORES = 8

P = 128
KD = DIM // P           # 8 k-chunks over D
SPLIT = 2               # cores per expert pair
EL = 2                  # experts per core (slots)
HS = HID // SPLIT       # 2048 H-rows per core per expert
KHS = HS // P           # 16 kk chunks (phase-2 contraction per slot)
JB = 4                  # j-tiles per streamed weight block
NPAIRS = JB // 2
GPE = 2 * HS // (P * JB)    # 8 blocks per slot
GROUPS_T = EL * GPE         # 16

_compiled = {}
_weight_cache = {"key": None, "packed": None}


def _chunks(total, step):
    out, c0 = [], 0
    while c0 < total:
        out.append((c0, min(step, total - c0)))
        c0 += step
    return out


def _build(segs, reps=1):
    """segs: per-slot padded token counts, e.g. (544, 512)."""
    segs = tuple(int(s) for s in segs)
    CT = sum(segs)
    offs = [0, segs[0]]

    nc = bacc.Bacc("TRN2", target_bir_lowering=False, debug=False,
                   num_devices=NCORES)
    dt = mybir.dt
    wA = nc.dram_tensor("wA", [GROUPS_T, P, KD, JB * P], dt.bfloat16,
                        kind="ExternalInput")
    w2 = nc.dram_tensor("w2", [P, EL * KHS, DIM], dt.bfloat16,
                        kind="ExternalInput")
    xT = nc.dram_tensor("xT", [P, KD, CT], dt.bfloat16, kind="ExternalInput")
    y = nc.dram_tensor("y", [DIM, CT], dt.float32, kind="ExternalOutput")

    with tile.TileContext(nc) as tc:
        with (
            tc.tile_pool(name="resident", bufs=1) as resident,
            tc.tile_pool(name="wstream", bufs=3) as wstream,
            tc.tile_pool(name="hgp", bufs=2) as hgpool,
            tc.tile_pool(name="gtmp", bufs=2) as gpool,
            tc.tile_pool(name="yout", bufs=3) as ypool,
            tc.tile_pool(name="ps1", bufs=5, space="PSUM") as ps1,
            tc.tile_pool(name="ps2", bufs=3, space="PSUM") as ps2,
        ):
          for _rep in range(reps):
            xsb = resident.tile([P, KD, CT], dt.bfloat16, tag="xsb")
            w2sb = resident.tile([P, EL * KHS, DIM], dt.bfloat16, tag="w2sb")

            for le in range(EL):
                off, Ce = offs[le], segs[le]
                cchunks = _chunks(Ce, 512)
                hg = hgpool.tile([P, KHS, Ce], dt.bfloat16, tag="hg")
                for gb in range(GPE):
                    g = le * GPE + gb
                    wsb = wstream.tile([P, KD, JB * P], dt.bfloat16,
                                       tag="wsb")
                    if g == 0:
                        # startup: interleave weight-k and x-slot0-k chunks;
                        # x rides the Activation HWDGE queue
                        for k in range(KD):
                            nc.sync.dma_start(wsb[:, k, :], wA[0, :, k, :])
                            nc.scalar.dma_start(
                                xsb[:, k, off:off + Ce],
                                xT[:, k, off:off + Ce])
                    else:
                        nc.sync.dma_start(wsb[:], wA[g])
                    # w2 slot prefetch, paced 4 kk-chunks per mid block
                    # (away from the startup-critical early queue)
                    if 2 <= gb < 2 + KHS // 4:
                        kk0 = le * KHS + (gb - 2) * 4
                        nc.sync.dma_start(w2sb[:, kk0:kk0 + 4, :],
                                          w2[:, kk0:kk0 + 4, :])
                    # second slot's token segment, well ahead of its use
                    if gb == 1 and le == 0 and EL > 1:
                        o2, C2 = offs[1], segs[1]
                        nc.scalar.dma_start(xsb[:, :, o2:o2 + C2],
                                            xT[:, :, o2:o2 + C2])
                    if g == 0:
                        # k-outer over all 4 psum groups: each arriving
                        # k-chunk DMA feeds 8 matmuls (startup cadence)
                        for (c0, cn) in cchunks:
                            ps = [ps1.tile([P, cn], dt.float32, tag="ps_h",
                                           name=f"ps{i}")
                                  for i in range(2 * NPAIRS)]
                            for k in range(KD):
                                for jt in range(NPAIRS):
                                    nc.tensor.matmul(
                                        ps[2 * jt][:],
                                        wsb[:, k, jt * P:(jt + 1) * P],
                                        xsb[:, k, off + c0:off + c0 + cn],
                                        start=(k == 0), stop=(k == KD - 1))
                                    nc.tensor.matmul(
                                        ps[2 * jt + 1][:],
                                        wsb[:, k,
                                            (NPAIRS + jt) * P:(NPAIRS + jt + 1) * P],
                                        xsb[:, k, off + c0:off + c0 + cn],
                                        start=(k == 0), stop=(k == KD - 1))
                            for jt in range(NPAIRS):
                                gt = gpool.tile([P, cn], dt.float32,
                                                tag="gt")
                                nc.scalar.activation(
                                    gt[:], ps[2 * jt + 1][:],
                                    mybir.ActivationFunctionType.Silu)
                                nc.vector.tensor_mul(
                                    hg[:, jt, c0:c0 + cn],
                                    ps[2 * jt][:], gt[:])
                        continue
                    for jt in range(NPAIRS):
                        jl = gb * NPAIRS + jt   # hg row-tile, 0..KHS-1
                        for (c0, cn) in cchunks:
                            ps_h = ps1.tile([P, cn], dt.float32, tag="ps_h")
                            ps_g = ps1.tile([P, cn], dt.float32, tag="ps_h")
                            for k in range(KD):
                                nc.tensor.matmul(
                                    ps_h[:], wsb[:, k, jt * P:(jt + 1) * P],
                                    xsb[:, k, off + c0:off + c0 + cn],
                                    start=(k == 0), stop=(k == KD - 1))
                            for k in range(KD):
                                nc.tensor.matmul(
                                    ps_g[:],
                                    wsb[:, k,
                                        (NPAIRS + jt) * P:(NPAIRS + jt + 1) * P],
                                    xsb[:, k, off + c0:off + c0 + cn],
                                    start=(k == 0), stop=(k == KD - 1))
                            gt = gpool.tile([P, cn], dt.float32, tag="gt")
                            nc.scalar.activation(
                                gt[:], ps_g[:],
                                mybir.ActivationFunctionType.Silu)
                            nc.vector.tensor_mul(
                                hg[:, jl, c0:c0 + cn], ps_h[:], gt[:])

                # phase 2 for this slot: y^T partial over the H-slice
                for dtile in range(DIM // P):
                    ysb = ypool.tile([P, Ce], dt.float32, tag="ysb")
                    for (c0, cn) in cchunks:
                        ps_y = ps2.tile([P, cn], dt.float32, tag="ps_y")
                        for a in range(KHS):
                            nc.tensor.matmul(
                                ps_y[:],
                                w2sb[:, le * KHS + a,
                                     dtile * P:(dtile + 1) * P],
                                hg[:, a, c0:c0 + cn],
                                start=(a == 0), stop=(a == KHS - 1))
                        nc.vector.tensor_copy(ysb[:, c0:c0 + cn], ps_y[:])
                    # one merged output DMA per d-tile, on the ACT queue
                    nc.scalar.dma_start(
                        y[dtile * P:(dtile + 1) * P, off:off + Ce], ysb[:])

    nc.compile()
    return nc


def _route(x2d, Wr):
    logits = (x2d @ Wr.T.astype(np.float32)).astype(np.float32)
    order = np.argsort(-logits, axis=1, kind="stable")
    top_idx = order[:, :TOP_K]
    top_val = np.take_along_axis(logits, top_idx, axis=1)
    m = top_val.max(axis=1, keepdims=True)
    w = np.exp(top_val - m, dtype=np.float32)
    top_w = (w / w.sum(axis=1, keepdims=True)).astype(np.float32)

    z_loss = np.float32(np.mean(np.square(logits), dtype=np.float32)
                        * Z_LOSS_COEF)
    lm = logits.max(axis=1, keepdims=True)
    p = np.exp(logits - lm, dtype=np.float32)
    probs = p / p.sum(axis=1, keepdims=True)
    pmean = probs.mean(axis=0, dtype=np.float32)
    balance_loss = np.float32(
        np.mean(np.square(pmean - np.float32(1.0 / E)), dtype=np.float32))
    return top_idx, top_w, z_loss, balance_loss


def _pack_w13(W1s, W3s):
    """[HS,D] fp32 slice pair -> [GPE, P, KD, JB*P] bf16 blocks."""
    kh = HS // P
    A = np.ascontiguousarray(W1s.T).reshape(KD, P, kh, P)
    B = np.ascontiguousarray(W3s.T).reshape(KD, P, kh, P)
    Ag = A.reshape(KD, P, GPE, NPAIRS, P).transpose(2, 1, 0, 3, 4)
    Bg = B.reshape(KD, P, GPE, NPAIRS, P).transpose(2, 1, 0, 3, 4)
    blk = np.concatenate(
        [Ag.reshape(GPE, P, KD, NPAIRS * P),
         Bg.reshape(GPE, P, KD, NPAIRS * P)], axis=3)
    return np.ascontiguousarray(blk).astype(BF16)


def kernel(x, Wr, W1, W2, W3):
    x = np.asarray(x, dtype=np.float32)
    Wr = np.asarray(Wr, dtype=np.float32)
    W1 = np.asarray(W1, dtype=np.float32)
    W2 = np.asarray(W2, dtype=np.float32)
    W3 = np.asarray(W3, dtype=np.float32)

    Bb, S, D = x.shape
    T = Bb * S
    x2d = np.ascontiguousarray(x.reshape(T, D))

    top_idx, top_w, z_loss, balance_loss = _route(x2d, Wr)

    idx_e, w_e = [], []
    for e in range(E):
        hits = np.nonzero(top_idx == e)
        idx_e.append(hits[0].astype(np.int64))
        w_e.append(top_w[hits[0], hits[1]].astype(np.float32))
    counts = np.array([len(i) for i in idx_e])
    padded = np.maximum(32, np.ceil(counts / 32).astype(int) * 32)

    # pair the 4 heaviest experts (slot 0) with the 4 lightest (slot 1)
    order = np.argsort(-padded, kind="stable")
    slot0 = [int(order[p]) for p in range(4)]
    slot1 = [int(order[7 - p]) for p in range(4)]
    segs = (int(padded[slot0].max()), int(padded[slot1].max()))
    offs = [0, segs[0]]
    CT = sum(segs)

    if segs not in _compiled:
        _compiled[segs] = _build(segs)
    nc = _compiled[segs]

    # per-pair x^T (both cores of a pair see the same tokens)
    xTs = []
    for p in range(4):
        xg = np.zeros((CT, D), dtype=np.float32)
        for sl, e in ((0, slot0[p]), (1, slot1[p])):
            xg[offs[sl]:offs[sl] + counts[e]] = x2d[idx_e[e]]
        xTs.append(np.ascontiguousarray(
            xg.reshape(CT, KD, P).transpose(2, 1, 0)).astype(BF16))

    wkey = (W1, W2, W3)
    if (_weight_cache["key"] is not None
            and all(np.array_equal(a, b) for a, b in
                    zip(_weight_cache["key"], wkey))
            and _weight_cache["slots"] == (tuple(slot0), tuple(slot1))):
        packed = _weight_cache["packed"]
    else:
        packed = []
        for c in range(NCORES):
            p, half = c // 2, c % 2
            r0, r1 = half * HS, (half + 1) * HS
            exps = (slot0[p], slot1[p])
            wAc = np.concatenate(
                [_pack_w13(W1[e][r0:r1], W3[e][r0:r1]) for e in exps],
                axis=0)
            w2c = np.concatenate(
                [np.ascontiguousarray(W2[e][:, r0:r1].T)
                 .reshape(KHS, P, DIM).transpose(1, 0, 2)
                 for e in exps], axis=1)
            packed.append({"wA": np.ascontiguousarray(wAc),
                           "w2": np.ascontiguousarray(w2c).astype(BF16)})
        _weight_cache["key"] = tuple(a.copy() for a in wkey)
        _weight_cache["slots"] = (tuple(slot0), tuple(slot1))
        _weight_cache["packed"] = packed

    in_maps = [{"wA": packed[c]["wA"], "w2": packed[c]["w2"],
                "xT": xTs[c // 2]} for c in range(NCORES)]

    res = None
    for attempt in range(3):
        try:
            res = run_bass_kernel_spmd(nc, in_maps,
                                       core_ids=list(range(NCORES)))
            break
        except Exception:
            if attempt == 2:
                raise
            import time as _time
            _time.sleep(2.0)

    y_full = np.zeros((T, D), dtype=np.float32)
    for p in range(4):
        ysum = res.results[2 * p]["y"] + res.results[2 * p + 1]["y"]
        for sl, e in ((0, slot0[p]), (1, slot1[p])):
            n = counts[e]
            if n:
                y_full[idx_e[e]] += (ysum[:, offs[sl]:offs[sl] + n].T
                                     * w_e[e][:, None])

    return (y_full.reshape(Bb, S, D), z_loss, balance_loss)


# revision 11
# speedup vs baseline: 1.0301x; 1.0030x over previous
"""MoE kernel v2: pairwise H-sliced expert parallelism.

Core pairs (2p, 2p+1) jointly own two experts: each core holds H-rows
[(c%2)*2048, (c%2+1)*2048) of both experts' W1/W3 (and the matching W2
columns) and processes both experts' exact token lists. Per-core slot
capacities are (max big-expert, max small-expert) over pairs -- (544, 512)
here vs a uniform 544+... in the expert-per-core layout: PE columns drop
418k -> 405k and every core does identical work. The host sums each pair's
two partial y^T arrays (fp32) and scatter-adds with the routing coefs.
"""

import numpy as np
import ml_dtypes

import concourse.bacc as bacc
import concourse.mybir as mybir
import concourse.tile as tile
from concourse.bass_utils import run_bass_kernel_spmd

BF16 = ml_dtypes.bfloat16

DIM = 1024
HID = 4096
E = 8
TOP_K = 2
Z_LOSS_COEF = 0.001
NCORES = 8

P = 128
KD = DIM // P           # 8 k-chunks over D
SPLIT = 2               # cores per expert pair
EL = 2                  # experts per core (slots)
HS = HID // SPLIT       # 2048 H-rows per core per expert
KHS = HS // P           # 16 kk chunks (phase-2 contraction per slot)
JB = 4                  # j-tiles per streamed weight block
NPAIRS = JB // 2
GPE = 2 * HS // (P * JB)    # 8 blocks per slot
GROUPS_T = EL * GPE         # 16

_compiled = {}
_weight_cache = {"key": None, "packed": None}


def _chunks(total, step):
    out, c0 = [], 0
    while c0 < total:
        out.append((c0, min(step, total - c0)))
        c0 += step
    return out


def _build(segs, reps=1):
    """segs: per-slot padded token counts, e.g. (544, 512)."""
    segs = tuple(int(s) for s in segs)
    CT = sum(segs)
    offs = [0, segs[0]]

    nc = bacc.Bacc("TRN2", target_bir_lowering=False, debug=False,
                   num_devices=NCORES)
    dt = mybir.dt
    wA = nc.dram_tensor("wA", [GROUPS_T, P, KD, JB * P], dt.bfloat16,
                        kind="ExternalInput")
    w2 = nc.dram_tensor("w2", [P, EL * KHS, DIM], dt.bfloat16,
                        kind="ExternalInput")
    xT = nc.dram_tensor("xT", [P, KD, CT], dt.bfloat16, kind="ExternalInput")
    y = nc.dram_tensor("y", [DIM, CT], dt.float32, kind="ExternalOutput")

    with tile.TileContext(nc) as tc:
        with (
            tc.tile_pool(name="resident", bufs=1) as resident,
            tc.tile_pool(name="wstream", bufs=3) as wstream,
            tc.tile_pool(name="hgp", bufs=2) as hgpool,
            tc.tile_pool(name="gtmp", bufs=2) as gpool,
            tc.tile_pool(name="yout", bufs=3) as ypool,
            tc.tile_pool(name="ps1", bufs=5, space="PSUM") as ps1,
            tc.tile_pool(name="ps2", bufs=3, space="PSUM") as ps2,
        ):
          for _rep in range(reps):
            xsb = resident.tile([P, KD, CT], dt.bfloat16, tag="xsb")
            w2sb = resident.tile([P, EL * KHS, DIM], dt.bfloat16, tag="w2sb")

            for le in range(EL):
                off, Ce = offs[le], segs[le]
                cchunks = _chunks(Ce, 512)
                hg = hgpool.tile([P, KHS, Ce], dt.bfloat16, tag="hg")
                for gb in range(GPE):
                    g = le * GPE + gb
                    wsb = wstream.tile([P, KD, JB * P], dt.bfloat16,
                                       tag="wsb")
                    if g == 0:
                        # startup: interleave weight-k and x-slot0-k chunks;
                        # x rides the Activation HWDGE queue
                        for k in range(KD):
                            nc.sync.dma_start(wsb[:, k, :], wA[0, :, k, :])
                            nc.scalar.dma_start(
                                xsb[:, k, off:off + Ce],
                                xT[:, k, off:off + Ce])
                    else:
                        nc.sync.dma_start(wsb[:], wA[g])
                    # w2 slot prefetch, paced 4 kk-chunks per mid block
                    # (away from the startup-critical early queue)
                    if 2 <= gb < 2 + KHS // 4:
                        kk0 = le * KHS + (gb - 2) * 4
                        nc.sync.dma_start(w2sb[:, kk0:kk0 + 4, :],
                                          w2[:, kk0:kk0 + 4, :])
                    # second slot's token segment, well ahead of its use
                    if gb == 1 and le == 0 and EL > 1:
                        o2, C2 = offs[1], segs[1]
                        nc.scalar.dma_start(xsb[:, :, o2:o2 + C2],
                                            xT[:, :, o2:o2 + C2])
                    if g == 0:
                        # k-outer over all 4 psum groups: each arriving
                        # k-chunk DMA feeds 8 matmuls (startup cadence)
                        for (c0, cn) in cchunks:
                            ps = [ps1.tile([P, cn], dt.float32, tag="ps_h",
                                           name=f"ps{i}")
                                  for i in range(2 * NPAIRS)]
                            for k in range(KD):
                                for jt in range(NPAIRS):
                                    nc.tensor.matmul(
                                        ps[2 * jt][:],
                                        wsb[:, k, jt * P:(jt + 1) * P],
                                        xsb[:, k, off + c0:off + c0 + cn],
                                        start=(k == 0), stop=(k == KD - 1))
                                    nc.tensor.matmul(
                                        ps[2 * jt + 1][:],
                                        wsb[:, k,
                                            (NPAIRS + jt) * P:(NPAIRS + jt + 1) * P],
                                        xsb[:, k, off + c0:off + c0 + cn],
                                        start=(k == 0), stop=(k == KD - 1))
                            for jt in range(NPAIRS):
                                gt = gpool.tile([P, cn], dt.float32,
                                                tag="gt")
                                nc.scalar.activation(
                                    gt[:], ps[2 * jt + 1][:],
                                    mybir.ActivationFunctionType.Silu)
                                nc.vector.tensor_mul(
                                    hg[:, jt, c0:c0 + cn],
                                    ps[2 * jt][:], gt[:])
                        continue
                    for jt in range(NPAIRS):
                        jl = gb * NPAIRS + jt   # hg row-tile, 0..KHS-1
                        for (c0, cn) in cchunks:
                            ps_h = ps1.tile([P, cn], dt.float32, tag="ps_h")
                            ps_g = ps1.tile([P, cn], dt.float32, tag="ps_h")
                            for k in range(KD):
                                nc.tensor.matmul(
                                    ps_h[:], wsb[:, k, jt * P:(jt + 1) * P],
                                    xsb[:, k, off + c0:off + c0 + cn],
                                    start=(k == 0), stop=(k == KD - 1))
                            for k in range(KD):
                                nc.tensor.matmul(
                                    ps_g[:],
                                    wsb[:, k,
                                        (NPAIRS + jt) * P:(NPAIRS + jt + 1) * P],
                                    xsb[:, k, off + c0:off + c0 + cn],
                                    start=(k == 0), stop=(k == KD - 1))
                            gt = gpool.tile([P, cn], dt.float32, tag="gt")
                            nc.scalar.activation(
                                gt[:], ps_g[:],
                                mybir.ActivationFunctionType.Silu)
                            nc.vector.tensor_mul(
                                hg[:, jl, c0:c0 + cn], ps_h[:], gt[:])

                # phase 2 for this slot: y^T partial over the H-slice
                for dtile in range(DIM // P):
                    ysb = ypool.tile([P, Ce], dt.float32, tag="ysb")
                    last = (le == EL - 1 and dtile == DIM // P - 1)
                    for (c0, cn) in (_chunks(Ce, 256) if last else cchunks):
                        ps_y = ps2.tile([P, cn], dt.float32, tag="ps_y")
                        for a in range(KHS):
                            nc.tensor.matmul(
                                ps_y[:],
                                w2sb[:, le * KHS + a,
                                     dtile * P:(dtile + 1) * P],
                                hg[:, a, c0:c0 + cn],
                                start=(a == 0), stop=(a == KHS - 1))
                        nc.vector.tensor_copy(ysb[:, c0:c0 + cn], ps_y[:])
                    # one merged output DMA per d-tile, on the ACT queue
                    nc.scalar.dma_start(
                        y[dtile * P:(dtile + 1) * P, off:off + Ce], ysb[:])

    nc.compile()
    return nc


def _route(x2d, Wr):
    logits = (x2d @ Wr.T.astype(np.float32)).astype(np.float32)
    order = np.argsort(-logits, axis=1, kind="stable")
    top_idx = order[:, :TOP_K]
    top_val = np.take_along_axis(logits, top_idx, axis=1)
    m = top_val.max(axis=1, keepdims=True)
    w = np.exp(top_val - m, dtype=np.float32)
    top_w = (w / w.sum(axis=1, keepdims=True)).astype(np.float32)

    z_loss = np.float32(np.mean(np.square(logits), dtype=np.float32)
                        * Z_LOSS_COEF)
    lm = logits.max(axis=1, keepdims=True)
    p = np.exp(logits - lm, dtype=np.float32)
    probs = p / p.sum(axis=1, keepdims=True)
    pmean = probs.mean(axis=0, dtype=np.float32)
    balance_loss = np.float32(
        np.mean(np.square(pmean - np.float32(1.0 / E)), dtype=np.float32))
    return top_idx, top_w, z_loss, balance_loss


def _pack_w13(W1s, W3s):
    """[HS,D] fp32 slice pair -> [GPE, P, KD, JB*P] bf16 blocks."""
    kh = HS // P
    A = np.ascontiguousarray(W1s.T).reshape(KD, P, kh, P)
    B = np.ascontiguousarray(W3s.T).reshape(KD, P, kh, P)
    Ag = A.reshape(KD, P, GPE, NPAIRS, P).transpose(2, 1, 0, 3, 4)
    Bg = B.reshape(KD, P, GPE, NPAIRS, P).transpose(2, 1, 0, 3, 4)
    blk = np.concatenate(
        [Ag.reshape(GPE, P, KD, NPAIRS * P),
         Bg.reshape(GPE, P, KD, NPAIRS * P)], axis=3)
    return np.ascontiguousarray(blk).astype(BF16)


def kernel(x, Wr, W1, W2, W3):
    x = np.asarray(x, dtype=np.float32)
    Wr = np.asarray(Wr, dtype=np.float32)
    W1 = np.asarray(W1, dtype=np.float32)
    W2 = np.asarray(W2, dtype=np.float32)
    W3 = np.asarray(W3, dtype=np.float32)

    Bb, S, D = x.shape
    T = Bb * S
    x2d = np.ascontiguousarray(x.reshape(T, D))

    top_idx, top_w, z_loss, balance_loss = _route(x2d, Wr)

    idx_e, w_e = [], []
    for e in range(E):
        hits = np.nonzero(top_idx == e)
        idx_e.append(hits[0].astype(np.int64))
        w_e.append(top_w[hits[0], hits[1]].astype(np.float32))
    counts = np.array([len(i) for i in idx_e])
    padded = np.maximum(32, np.ceil(counts / 4).astype(int) * 4)

    # pair the 4 heaviest experts (slot 0) with the 4 lightest (slot 1)
    order = np.argsort(-padded, kind="stable")
    slot0 = [int(order[p]) for p in range(4)]
    slot1 = [int(order[7 - p]) for p in range(4)]
    segs = (int(padded[slot0].max()), int(padded[slot1].max()))
    offs = [0, segs[0]]
    CT = sum(segs)

    if segs not in _compiled:
        _compiled[segs] = _build(segs)
    nc = _compiled[segs]

    # per-pair x^T (both cores of a pair see the same tokens)
    xTs = []
    for p in range(4):
        xg = np.zeros((CT, D), dtype=np.float32)
        for sl, e in ((0, slot0[p]), (1, slot1[p])):
            xg[offs[sl]:offs[sl] + counts[e]] = x2d[idx_e[e]]
        xTs.append(np.ascontiguousarray(
            xg.reshape(CT, KD, P).transpose(2, 1, 0)).astype(BF16))

    wkey = (W1, W2, W3)
    if (_weight_cache["key"] is not None
            and all(np.array_equal(a, b) for a, b in
                    zip(_weight_cache["key"], wkey))
            and _weight_cache["slots"] == (tuple(slot0), tuple(slot1))):
        packed = _weight_cache["packed"]
    else:
        packed = []
        for c in range(NCORES):
            p, half = c // 2, c % 2
            r0, r1 = half * HS, (half + 1) * HS
            exps = (slot0[p], slot1[p])
            wAc = np.concatenate(
                [_pack_w13(W1[e][r0:r1], W3[e][r0:r1]) for e in exps],
                axis=0)
            w2c = np.concatenate(
                [np.ascontiguousarray(W2[e][:, r0:r1].T)
                 .reshape(KHS, P, DIM).transpose(1, 0, 2)
                 for e in exps], axis=1)
            packed.append({"wA": np.ascontiguousarray(wAc),
                           "w2": np.ascontiguousarray(w2c).astype(BF16)})
        _weight_cache["key"] = tuple(a.copy() for a in wkey)
        _weight_cache["slots"] = (tuple(slot0), tuple(slot1))
        _weight_cache["packed"] = packed

    in_maps = [{"wA": packed[c]["wA"], "w2": packed[c]["w2"],
                "xT": xTs[c // 2]} for c in range(NCORES)]

    res = None
    for attempt in range(3):
        try:
            res = run_bass_kernel_spmd(nc, in_maps,
                                       core_ids=list(range(NCORES)))
            break
        except Exception:
            if attempt == 2:
                raise
            import time as _time
            _time.sleep(2.0)

    y_full = np.zeros((T, D), dtype=np.float32)
    for p in range(4):
        ysum = res.results[2 * p]["y"] + res.results[2 * p + 1]["y"]
        for sl, e in ((0, slot0[p]), (1, slot1[p])):
            n = counts[e]
            if n:
                y_full[idx_e[e]] += (ysum[:, offs[sl]:offs[sl] + n].T
                                     * w_e[e][:, None])

    return (y_full.reshape(Bb, S, D), z_loss, balance_loss)
